# revision 2
# baseline (speedup 1.0000x reference)
"""Trainium2 Bass kernel for nn_DiffusionModel_56822417326086 (v2).

Causal multi-head self-attention block:
    qkv = x @ w_qkv ; split into 8 heads of 64
    e = (q @ k^T) * DH^-0.5 ; causal mask ; a = softmax(e)
    o = a @ v ; y = o @ w_out + b_out ; y *= m

Sharding (8 cores, zero collectives):
    core c -> batch b = c // 2, head-quad q = c % 2 (heads 4q..4q+3).
    Host sums the two partial projections per batch, adds b_out, applies m.
    (m is all-ones per the spec; the device kernel assumes pure causal.)

v2 changes over the 216us baseline (trace-driven):
  - The normalize + output-projection work is EMITTED INTERLEAVED inside
    the next query-chunk's key-block loop, so it runs under attention
    instead of as a 60us serial tail (which also HAM-throttled the PE
    to 4/8 clock for ~78us of the run).
  - Causal narrowing: for the 4 diagonal key-blocks of each query chunk
    the scores / exp / mask / A@V all operate only on the valid query
    range (scores keep M>=256 to dodge the f32r small-M penalty).
    Saves ~15us ACT, ~8us PE, ~10us DVE.
  - oA/oB merged into one 2-bank PSUM tile so the softmax denominators
    (65th "ones" row of V) are read by ONE reciprocal op straight from
    PSUM row 64; bcast rhs comes back via a single DMA.
  - k-copies ride the ACT engine (idle pre-attention), q-copies on DVE.
  - exp tiles [128,2,512]: one ACT op covers both heads of a pair
    (ACT per-op overhead ~200ns).
"""

import numpy as np
import ml_dtypes
from contextlib import ExitStack

B, T, D, H = 4, 2048, 512, 8
DH = D // H
SCALE = DH ** -0.5
QC = 512           # query-chunk
NQC = T // QC      # 4
KB = 128           # key-block

_CACHE = {}


def _build_program():
    import concourse.mybir as mybir
    import concourse.tile as tile
    from concourse import bacc

    f32 = mybir.dt.float32
    f32r = mybir.dt.float32r
    bf16 = mybir.dt.bfloat16
    Exp = mybir.ActivationFunctionType.Exp

    nc = bacc.Bacc("TRN2", target_bir_lowering=False, debug=False)

    xT_d = nc.dram_tensor("xT", [D, T], bf16, kind="ExternalInput").ap()
    wq_d = nc.dram_tensor("wq2", [2, D, 128], bf16, kind="ExternalInput").ap()
    wk_d = nc.dram_tensor("wk2", [2, D, 128], bf16, kind="ExternalInput").ap()
    wv_d = nc.dram_tensor("wv4", [D, 256], bf16, kind="ExternalInput").ap()
    wo_d = nc.dram_tensor("wo4", [256, D], bf16, kind="ExternalInput").ap()
    dm_d = nc.dram_tensor("dmst", [128, 256], bf16, kind="ExternalInput").ap()
    y_d = nc.dram_tensor("y", [T, D], f32, kind="ExternalOutput").ap()

    with tile.TileContext(nc) as tc, ExitStack() as ctx:
        consts = ctx.enter_context(tc.tile_pool(name="consts", bufs=1))
        work = ctx.enter_context(tc.tile_pool(name="work", bufs=2))
        yt_pool = ctx.enter_context(tc.tile_pool(name="ytp", bufs=3))
        exp_pool = ctx.enter_context(tc.tile_pool(name="exp", bufs=6))
        ps = ctx.enter_context(tc.tile_pool(name="ps", bufs=2, space="PSUM"))
        oab_pool = ctx.enter_context(tc.tile_pool(name="oab", bufs=2, space="PSUM"))

        # ---- persistent tiles ----------------------------------------------
        qT2 = consts.tile([128, 2, T], bf16)  # partitions 0-63 head A, 64-127 B
        kT2 = consts.tile([128, 2, T], bf16)
        vsb = consts.tile([128, 16, 4, 65], bf16)
        wo = consts.tile([128, 2, D], bf16)
        dm = consts.tile([128, 2, 128], bf16)
        oUA = consts.tile([64, 2, T], f32)
        oUB = consts.tile([64, 2, T], f32)
        oTn2 = consts.tile([128, 2, T], bf16)
        ones64f = consts.tile([1, 64], f32)
        ones64 = consts.tile([1, 64], bf16)
        ones128 = consts.tile([1, 128], bf16)
        warm = consts.tile([1, 512], bf16)

        nc.vector.memset(ones64f[:], 1.0)
        nc.vector.tensor_copy(ones64[:], ones64f[:])
        nc.vector.tensor_copy(ones128[0:1, 0:64], ones64f[:])
        nc.vector.tensor_copy(ones128[0:1, 64:128], ones64f[:])
        nc.vector.memset(vsb[:, :, :, 64:65], 1.0)
        for _w in range(8):
            nc.vector.tensor_copy(warm[0:1, _w * 64:(_w + 1) * 64], ones64f[:])

        # small consts + output-proj weights early on the SWDGE queue
        nc.gpsimd.dma_start(dm[:], dm_d)
        for pp in range(2):
            nc.gpsimd.dma_start(wo[:, pp, :], wo_d[pp * 128:(pp + 1) * 128, :])

        # PE warm-up: release the HAM clock gate while input DMAs stream
        for _ in range(6):
            wps = ps.tile([64, 512], f32, tag="sc")
            nc.tensor.matmul(wps[:], ones64[:],
                             warm[:], start=True, stop=True)

        # ---- state shared between emission hooks ---------------------------
        st = {}

        def n_oU(p, qc):
            """Drain oAB(p,qc): unnormalized o to SBUF + reciprocal of sums.

            The reciprocal is ~6.4ns/elem/lane, so 1024 sums on the single
            PSUM row would hog the in-order DVE queue for 6.5us.  Instead:
            1-lane copy out (1 cyc/elem), DMA-scatter across 128 partitions,
            reciprocal at 8 elems/lane, DMA-gather back for the bcast rhs.
            """
            qsl = slice(qc * QC, (qc + 1) * QC)
            oAB = st[("oab", p, qc)]
            nc.vector.tensor_copy(oUA[:, p, qsl], oAB[0:64, 0, :])
            nc.vector.tensor_copy(oUB[:, p, qsl], oAB[0:64, 1, :])
            rec = work.tile([128, 2, 512], f32, tag="rec")
            nc.vector.tensor_copy(rec[64:65, :, :], oAB[64:65, :, :])
            s128 = work.tile([128, 8], f32, tag="s128")
            nc.sync.dma_start(s128[:], rec[64:65, :, :])
            r128f = work.tile([128, 8], f32, tag="r128f")
            nc.vector.reciprocal(r128f[:], s128[:])
            r128 = work.tile([128, 8], bf16, tag="r128")
            with nc.allow_low_precision(reason="bf16 recip feeds bf16 bcast matmul"):
                nc.vector.tensor_copy(r128[:], r128f[:])
            recT = work.tile([1, 1024], bf16, tag="recT")
            nc.sync.dma_start(recT[:], r128[:])
            st[("recT", p, qc)] = recT

        def n_bc(p, qc):
            """Broadcast reciprocals across 64 partitions via K=1 matmuls."""
            recT = st[("recT", p, qc)]
            bc = ps.tile([64, 2, 512], f32, tag="sc")
            nc.tensor.matmul(bc[:, 0, :], ones64[:],
                             recT[0:1, 0:512], start=True, stop=True)
            nc.tensor.matmul(bc[:, 1, :], ones64[:],
                             recT[0:1, 512:1024], start=True, stop=True)
            st[("bc", p, qc)] = bc

        def n_mul(p, qc):
            """Normalize o; head B goes to partitions 64-127 via DMA shift."""
            qsl = slice(qc * QC, (qc + 1) * QC)
            bc = st[("bc", p, qc)]
            nc.vector.tensor_mul(oTn2[0:64, p, qsl], oUA[:, p, qsl], bc[:, 0, :])
            scrB = work.tile([64, 512], bf16, tag="scrB")
            nc.vector.tensor_mul(scrB[:], oUB[:, p, qsl], bc[:, 1, :])
            nc.sync.dma_start(oTn2[64:128, p, qsl], scrB[:])

        def n_out(qc, rcs, on_act=False):
            """Output projection for t-rows rcs (needs both pairs normalized)."""
            for rc in rcs:
                rsl = slice(rc * 128, (rc + 1) * 128)
                psy = ps.tile([128, 512], f32, tag="sc")
                nc.tensor.matmul(psy[:], oTn2[:, 0, rsl],
                                 wo[:, 0, :], start=True, stop=False)
                nc.tensor.matmul(psy[:], oTn2[:, 1, rsl],
                                 wo[:, 1, :], start=False, stop=True)
                yt = yt_pool.tile([128, 512], f32, tag="yt")
                if on_act:
                    nc.scalar.copy(yt[:], psy[:])
                    nc.scalar.dma_start(y_d[rsl, :], yt[:])
                else:
                    nc.vector.tensor_copy(yt[:], psy[:])
                    nc.gpsimd.dma_start(y_d[rsl, :], yt[:])

        def wfill(n):
            for _ in range(n):
                wps = ps.tile([64, 512], f32, tag="sc")
                nc.tensor.matmul(wps[:], ones64[:], warm[:], start=True, stop=True)

        def emit_av(item, oAB, p, nkb):
            kb, e0, ex = item
            nc.tensor.matmul(oAB[0:65, 0, e0:512], vsb[:, kb, 2 * p, :],
                             ex[:, 0, e0:512], start=kb == 0, stop=kb == nkb - 1)
            nc.tensor.matmul(oAB[0:65, 1, e0:512], vsb[:, kb, 2 * p + 1, :],
                             ex[:, 1, e0:512], start=kb == 0, stop=kb == nkb - 1)

        def attention(p, qc, hooks, fill=False):
            """One query-chunk of pair p, with emission hooks at kb indices.

            fill=True inserts one dependency-free filler matmul per key
            block (overwritten by the real scores) so the ACT-gated PE
            stream has no micro-idles for the HAM activity monitor to
            see -- otherwise the clock gate drops to 4/8 mid-attention.
            """
            nkb = 4 * (qc + 1)
            oAB = oab_pool.tile([128, 2, 512], f32, tag="o")
            st[("oab", p, qc)] = oAB
            avq = []
            for kb in range(nkb):
                v = kb - (nkb - 4)           # diagonal index (>=0 on diagonal)
                ksl = slice(kb * KB, (kb + 1) * KB)
                e0 = 0 if v < 1 else 128 * v           # first valid query col
                s0 = e0                                # bf16: no small-M penalty
                sps = ps.tile([128, 2, 512], f32, tag="sc")
                nc.tensor.matmul(
                    sps[:, 0, s0:512], kT2[0:64, p, ksl],
                    qT2[0:64, p, qc * QC + s0:(qc + 1) * QC],
                    start=True, stop=True, tile_position=(0, 0))
                nc.tensor.matmul(
                    sps[:, 1, s0:512], kT2[64:128, p, ksl],
                    qT2[64:128, p, qc * QC + s0:(qc + 1) * QC],
                    start=True, stop=True, tile_position=(64, 0))
                ex = exp_pool.tile([128, 2, 512], bf16, tag="exp")
                nc.scalar.activation(ex[:, :, e0:512], sps[:, :, e0:512],
                                     Exp, scale=SCALE)
                if v >= 0:
                    nc.vector.tensor_mul(ex[:, :, e0:e0 + 128],
                                         ex[:, :, e0:e0 + 128], dm[:])
                avq.append((kb, e0, ex))
                if len(avq) > 1:
                    emit_av(avq.pop(0), oAB, p, nkb)
                for fn in hooks.get(kb, ()):
                    fn()
            emit_av(avq.pop(0), oAB, p, nkb)
            for fn in hooks.get("end", ()):
                fn()

        # ---- qkv projection -------------------------------------------------
        with tc.tile_pool(name="qkvp", bufs=1) as qp:
            wq = qp.tile([128, 2, 4, 128], bf16)
            wk = qp.tile([128, 2, 4, 128], bf16)
            wv = qp.tile([128, 4, 256], bf16)
            xT = qp.tile([128, 4, T], bf16)

            # x is the startup long pole: its first column-wave goes out
            # before the (tiny) weight loads so qk can start ~9-10us in
            _eng = [nc.sync, nc.gpsimd, nc.scalar, nc.gpsimd]
            for kc in range(4):
                _eng[kc].dma_start(xT[:, kc, 0:512], xT_d[kc * 128:(kc + 1) * 128, 0:512])
            for kc in range(4):
                nc.scalar.dma_start(wq[:, 0, kc, :], wq_d[0, kc * 128:(kc + 1) * 128, :])
                nc.sync.dma_start(wk[:, 0, kc, :], wk_d[0, kc * 128:(kc + 1) * 128, :])
            for rc4 in range(1, 4):
                for kc in range(4):
                    _eng[kc].dma_start(
                        xT[:, kc, rc4 * 512:(rc4 + 1) * 512],
                        xT_d[kc * 128:(kc + 1) * 128, rc4 * 512:(rc4 + 1) * 512])
            for kc in range(4):
                nc.sync.dma_start(wv[:, kc, :], wv_d[kc * 128:(kc + 1) * 128, :])
                nc.scalar.dma_start(wq[:, 1, kc, :], wq_d[1, kc * 128:(kc + 1) * 128, :])
                nc.scalar.dma_start(wk[:, 1, kc, :], wk_d[1, kc * 128:(kc + 1) * 128, :])

            def qk_half(p, rc4, w, dst, on_act):
                """One q-or-k projection sub-group: 4 matmuls + 1 copy."""
                sl = slice(rc4 * 512, (rc4 + 1) * 512)
                pst = ps.tile([128, 512], f32, tag="sc")
                for kc in range(4):
                    nc.tensor.matmul(pst[:], w[:, p, kc, :], xT[:, kc, sl],
                                     start=kc == 0, stop=kc == 3)
                if on_act:
                    nc.scalar.copy(dst[:, p, sl], pst[:])
                else:
                    nc.vector.tensor_copy(dst[:, p, sl], pst[:])

            def v_proj(rcs):
                for rc in rcs:
                    psv = oab_pool.tile([128, 4, 64], f32, tag="o")
                    for kc in range(4):
                        nc.tensor.matmul(psv[:], xT[:, kc, rc * 128:(rc + 1) * 128],
                                         wv[:, kc, :], start=kc == 0, stop=kc == 3)
                    nc.vector.tensor_copy(vsb[:, rc, :, 0:64], psv[:])

            def Q1(rc4):
                return lambda: qk_half(1, rc4, wq, qT2, False)

            def K1(rc4):
                return lambda: qk_half(1, rc4, wk, kT2, False)

            for rc4 in range(4):
                qk_half(0, rc4, wq, qT2, False)
                qk_half(0, rc4, wk, kT2, False)
            v_proj(range(0, 4))

            # ---- attention; p1 qkv + normalize interleaved into the kb loops
            attention(0, 0, {})
            v_proj(range(4, 6))
            attention(0, 1, {0: [lambda: v_proj(range(6, 8))],
                             1: [Q1(0)],
                             2: [lambda: n_oU(0, 0)],
                             3: [K1(0)],
                             4: [lambda: n_bc(0, 0)],
                             5: [Q1(1)],
                             6: [lambda: n_mul(0, 0)],
                             7: [K1(1)]})
            attention(0, 2, {0: [lambda: n_oU(0, 1)],
                             1: [lambda: v_proj(range(8, 10))],
                             2: [lambda: v_proj(range(10, 12))],
                             3: [Q1(2)],
                             4: [lambda: n_bc(0, 1)],
                             5: [K1(2)],
                             6: [lambda: n_mul(0, 1)],
                             7: [Q1(3)],
                             9: [K1(3)]})
            attention(0, 3, {0: [lambda: n_oU(0, 2)],
                             1: [lambda: v_proj(range(12, 14))],
                             2: [lambda: v_proj(range(14, 16))],
                             4: [lambda: n_bc(0, 2)],
                             6: [lambda: n_mul(0, 2)],
                             "end": [lambda: n_oU(0, 3)]})

        attention(1, 0, {})
        attention(1, 1, {0: [lambda: n_bc(0, 3)],
                         1: [lambda: n_oU(1, 0)],
                         2: [lambda: n_mul(0, 3)],
                         5: [lambda: n_bc(1, 0)],
                         7: [lambda: n_mul(1, 0)]})
        attention(1, 2, {0: [lambda: n_oU(1, 1)],
                         4: [lambda: n_bc(1, 1)],
                         6: [lambda: n_mul(1, 1)],
                         9: [lambda: n_out(0, range(0, 2))],
                         10: [lambda: n_out(0, range(2, 4))]})
        attention(1, 3, {0: [lambda: n_oU(1, 2)],
                         4: [lambda: n_bc(1, 2)],
                         6: [lambda: n_mul(1, 2)],
                         9: [lambda: n_out(1, range(4, 6))],
                         10: [lambda: n_out(1, range(6, 8))],
                         12: [lambda: n_out(2, range(8, 10))],
                         14: [lambda: n_out(2, range(10, 12))],
                         "end": [lambda: n_oU(1, 3)]})
        # tail: last chunk's normalize + projection, with warm-fill matmuls
        # bridging the reciprocal chain so HAM stays at 8/8
        wfill(4)
        n_bc(1, 3)
        wfill(2)
        n_mul(1, 3)
        n_out(3, range(12, 16), on_act=True)

    nc.compile()
    return nc



def _diag_mask():
    j = np.arange(KB)[:, None]          # key within block
    i = np.arange(KB)[None, :]          # query within strip
    mv = np.where(i >= j, 1.0, 0.0).astype(np.float32)
    return np.tile(mv, (1, 2)).copy()   # duplicated for the head pair


def _prep_inputs(x, m, w_qkv, w_out):
    """Per-core input maps for SPMD dispatch."""
    dmst = _diag_mask().astype(ml_dtypes.bfloat16)
    wq_full = w_qkv[:, 0:D]
    wk_full = w_qkv[:, D:2 * D]
    wv_full = w_qkv[:, 2 * D:3 * D]
    in_maps = []
    for c in range(8):
        b, q = c // 2, c % 2
        hsl = slice(4 * q * DH, (4 * q + 4) * DH)
        wq2 = np.stack([
            np.concatenate([wq_full[:, (4 * q + 2 * p) * DH:(4 * q + 2 * p + 1) * DH],
                            wq_full[:, (4 * q + 2 * p + 1) * DH:(4 * q + 2 * p + 2) * DH]],
                           axis=1)
            for p in range(2)])
        wk2 = np.stack([
            np.concatenate([wk_full[:, (4 * q + 2 * p) * DH:(4 * q + 2 * p + 1) * DH],
                            wk_full[:, (4 * q + 2 * p + 1) * DH:(4 * q + 2 * p + 2) * DH]],
                           axis=1)
            for p in range(2)])
        in_maps.append({
            "xT": np.ascontiguousarray(x[b].T).astype(ml_dtypes.bfloat16),
            "wq2": np.ascontiguousarray(wq2).astype(ml_dtypes.bfloat16),
            "wk2": np.ascontiguousarray(wk2).astype(ml_dtypes.bfloat16),
            "wv4": np.ascontiguousarray(wv_full[:, hsl]).astype(ml_dtypes.bfloat16),
            "wo4": np.ascontiguousarray(w_out[hsl, :]).astype(ml_dtypes.bfloat16),
            "dmst": dmst,
        })
    return in_maps


def _execute(inputs, trace=False):
    from concourse.bass_utils import run_bass_kernel_spmd

    if "nc" not in _CACHE:
        _CACHE["nc"] = _build_program()
    nc = _CACHE["nc"]

    x = np.asarray(inputs["x"], np.float32)
    m = np.asarray(inputs["m"], np.float32)
    w_qkv = np.asarray(inputs["w_qkv"], np.float32)
    w_out = np.asarray(inputs["w_out"], np.float32)
    b_out = np.asarray(inputs["b_out"], np.float32)

    in_maps = _prep_inputs(x, m, w_qkv, w_out)
    res = run_bass_kernel_spmd(nc, in_maps, core_ids=list(range(8)), trace=trace)

    y = np.empty((B, T, D), np.float32)
    for b in range(B):
        y[b] = res.results[2 * b]["y"] + res.results[2 * b + 1]["y"]
    y += b_out[None, None, :]
    y *= m[..., None]
    return y, res


def kernel(**inputs) -> np.ndarray:
    y, _ = _execute(inputs, trace=False)
    return y
